# revision 1
# baseline (speedup 1.0000x reference)
"""v4: two-pass butterfly kernel (low 7 stages + high 3 stages), bf16 output.

Factor B = Bh @ Bl:
  Bl = stages 0..6  — block-diagonal over 8 contiguous 128-position blocks.
  Bh = stages 7..9  — mixes w = pos//128 across the 8 blocks, elementwise in
                      r = pos % 128.

Pass 1 (per 512-batch tile): y^T tiles in "q32" interleaved partition order.
  T[m] (m=0..3 r-range) [128, 2, 512] (h = w-half on the middle axis):
     partition p' = 32*wl + rl  <->  y position (32m + rl) + 128*(4h + wl)
  built by 8 column-packed matmuls (M=32, tile_position) with
  lhsT = Bl^T block slice [128, 32], rhs = x block [128, 512], into a
  2-bank PSUM tile; one [128, 1024] ScalarE eviction per m (bf16).

Pass 2 (per 128-batch chunk): out[b, :] batch-major, bf16.
  For each m-group (256 stored columns = (w_out, rl)):
     psum[:, mi, :] += T[m][h][:, chunk]^T @ D[m][h]   for h = 0, 1
  D[m][h][p', q] = Bh[pos_out, pos_in] (nonzero iff rl_out == rl_in).
  One DVE tensor_add per psum tile (+bias, [128, 2, 8, 32] scatter view)
  writes natural column order; half-tile (2-chunk) stores on the sync queue.
"""

import os
import sys
import numpy as np

for _p in ("/opt/trn_rl_repo", os.path.expanduser("~/.axon_site/_ro/trn_rl_repo")):
    if os.path.isdir(_p) and _p not in sys.path:
        sys.path.insert(0, _p)

import concourse.bass as bass
import concourse.bacc as bacc
import concourse.mybir as mybir
from concourse import tile
from concourse.bass_utils import run_bass_kernel_spmd

import ml_dtypes

N_CORES = 8
BATCH = 32768
N = 1024
LOG_N = 10
BC = BATCH // N_CORES   # 4096 rows per core
BT = 512                # batch tile (pass 1)
NBT = BC // BT          # 8
CHUNKS_PER_BT = BT // 128   # 4

_last_exec_time_ns = None
_nc_cache = None


def _apply_stages(m: np.ndarray, twiddle: np.ndarray, idxs) -> np.ndarray:
    """Apply butterfly stages `idxs` to the rows of m (batch of vectors)."""
    n = N
    for idx in idxs:
        s = 1 << idx
        g = n // (2 * s)
        t = twiddle[0, 0, idx].astype(np.float64).reshape(g, s, 2, 2)
        xr = m.reshape(-1, g, 2, s)
        m = np.einsum("grij,bgjr->bgir", t, xr).reshape(-1, n)
    return m


def _host_weights(twiddle: np.ndarray):
    eye = np.eye(N, dtype=np.float64)
    blt = _apply_stages(eye, twiddle, range(7))        # BlT[k, p] = Bl[p, k]
    bht = _apply_stages(eye, twiddle, range(7, 10))    # BhT[k, p] = Bh[p, k]

    # pass-1 lhsT: bl_pack[k, w, m, r32] = Bl[128w + 32m + r32, 128w + k]
    bl_pack = np.zeros((128, 8, 4, 32), dtype=np.float64)
    for w in range(8):
        blk = blt[128 * w:128 * (w + 1), 128 * w:128 * (w + 1)]  # [k, r]
        bl_pack[:, w] = blk.reshape(128, 4, 32)

    # pass-2 moving operand: d_pack[p', m, h, q]
    #   p' = 32*wl + rl_in  -> pos_in  = 32m + rl_in + 128*(4h + wl)
    #   q  = 32*w_out + rl_out -> pos_out = 32m + rl_out + 128*w_out
    # value = BhT[pos_in, pos_out]
    wl = np.arange(4)[:, None]          # [4, 1]
    rl = np.arange(32)[None, :]         # [1, 32]
    wo = np.arange(8)[:, None]
    d_pack = np.zeros((128, 4, 2, 256), dtype=np.float64)
    for m in range(4):
        for h in range(2):
            pos_in = (32 * m + rl + 128 * (4 * h + wl))        # [4, 32]
            pos_out = (32 * m + rl + 128 * wo)                 # [8, 32]
            # nonzero only when rl_in == rl_out
            sub = bht[np.ix_(pos_in.ravel(), pos_out.ravel())]  # [128, 256]
            mask = (rl.ravel()[None, :].repeat(4, 0).ravel()[:, None]
                    == rl.ravel()[None, :].repeat(8, 0).ravel()[None, :])
            d_pack[:, m, h, :] = np.where(mask, sub, 0.0)

    return bl_pack, d_pack


def _stored_bias(bias: np.ndarray) -> np.ndarray:
    # stored col s = m*256 + w*32 + r  ->  natural pos = 128w + 32m + r
    w = np.arange(8)
    m = np.arange(4)
    r = np.arange(32)
    pos = (128 * w[None, :, None] + 32 * m[:, None, None] + r[None, None, :])
    return np.ascontiguousarray(
        np.broadcast_to(bias[pos.ravel()].astype(np.float32), (128, N))
    )


def _build_nc():
    nc = bacc.Bacc("TRN2", target_bir_lowering=False)
    xtb = nc.dram_tensor("xtb", [NBT, 128, 8, BT], mybir.dt.bfloat16, kind="ExternalInput")
    bl = nc.dram_tensor("bl", [128, 8, 4, 32], mybir.dt.bfloat16, kind="ExternalInput")
    dd = nc.dram_tensor("dd", [128, 4, 2, 256], mybir.dt.bfloat16, kind="ExternalInput")
    bb = nc.dram_tensor("bb", [128, N], mybir.dt.float32, kind="ExternalInput")
    out = nc.dram_tensor("out", [NBT, 128, CHUNKS_PER_BT, N], mybir.dt.bfloat16,
                         kind="ExternalOutput")

    with tile.TileContext(nc) as tc:
        with (
            tc.tile_pool(name="const", bufs=1) as cpool,
            tc.tile_pool(name="tsb", bufs=12) as t_pool,
            tc.tile_pool(name="ot", bufs=3) as ot_pool,
            tc.tile_pool(name="ps1", bufs=4, space="PSUM") as ps1_pool,
            tc.tile_pool(name="ps2", bufs=4, space="PSUM") as ps2_pool,
        ):
            # pass-1 of batch tile 0 is gated only by bls + x tile 0; split
            # the first tile's load so the first matmuls start early
            bls = cpool.tile([128, 8, 4, 32], mybir.dt.bfloat16)
            nc.sync.dma_start(out=bls[:], in_=bl[:])

            xall = cpool.tile([128, 8, BC], mybir.dt.bfloat16)
            nc.sync.dma_start(out=xall[:, 0:2, 0:BT], in_=xtb[0][:, 0:2])
            nc.sync.dma_start(out=xall[:, 2:4, 0:BT], in_=xtb[0][:, 2:4])
            nc.sync.dma_start(out=xall[:, 4:8, 0:BT], in_=xtb[0][:, 4:8])

            dds = cpool.tile([128, 4, 2, 256], mybir.dt.bfloat16)
            nc.sync.dma_start(out=dds[:], in_=dd[:])
            bbt = cpool.tile([128, N], mybir.dt.float32)
            nc.sync.dma_start(out=bbt[:], in_=bb[:])

            for g in range(1, NBT):
                eng = nc.sync
                if g <= 2:
                    eng.dma_start(out=xall[:, 0:4, g * BT:(g + 1) * BT],
                                  in_=xtb[g][:, 0:4])
                    eng.dma_start(out=xall[:, 4:8, g * BT:(g + 1) * BT],
                                  in_=xtb[g][:, 4:8])
                else:
                    eng.dma_start(
                        out=xall[:, :, g * BT:(g + 1) * BT],
                        in_=xtb[g],
                    )

            def pass1(bt):
                bsl = slice(bt * BT, (bt + 1) * BT)
                tsb = {}
                for m in range(4):
                    t_t = t_pool.tile([128, 2, BT], mybir.dt.bfloat16)
                    for h in range(2):
                        ps = ps1_pool.tile([128, BT], mybir.dt.float32)
                        for wl in range(4):
                            w = 4 * h + wl
                            nc.tensor.matmul(
                                ps[32 * wl:32 * (wl + 1), :],
                                bls[:, w, m, :],
                                xall[:, w, bsl],
                                start=True,
                                stop=True,
                                tile_position=(0, 32 * wl),
                            )
                        nc.scalar.copy(out=t_t[:, h, :], in_=ps[:])
                    tsb[m] = t_t
                return tsb

            def pass2(bt, tsb):
                otg = ot_pool.tile([128, CHUNKS_PER_BT, N], mybir.dt.bfloat16)
                for cc in range(CHUNKS_PER_BT):
                    c0 = cc * 128
                    # per-m natural-order view: V[p, m, w, r] = ot[p, 128w+32m+r]
                    ot_v = otg[:, cc, :].rearrange("p (w m r) -> p m w r",
                                                   w=8, m=4, r=32)
                    for half in range(2):
                        ps2 = ps2_pool.tile([128, 2, 256], mybir.dt.float32)
                        for mi in range(2):
                            m = half * 2 + mi
                            for h in range(2):
                                nc.tensor.matmul(
                                    ps2[:, mi, :],
                                    tsb[m][:, h, c0:c0 + 128],
                                    dds[:, m, h, :],
                                    start=(h == 0),
                                    stop=(h == 1),
                                )
                        nc.vector.tensor_add(
                            ot_v[:, 2 * half:2 * half + 2],
                            ps2[:].rearrange("p m (w r) -> p m w r", w=8, r=32),
                            bbt[:, half * 512:(half + 1) * 512].rearrange(
                                "p (m w r) -> p m w r", m=2, w=8, r=32),
                        )
                    # stores ride the scalar engine's hardware DMA queue so
                    # the output stream does not share a ring with the loads
                    if bt == NBT - 1 and cc >= 2:
                        nc.sync.dma_start(out=out[bt][:, cc:cc + 1],
                                            in_=otg[:, cc:cc + 1])
                    elif cc == 1 or (cc == 3 and bt < NBT - 1):
                        nc.sync.dma_start(out=out[bt][:, cc - 1:cc + 1],
                                            in_=otg[:, cc - 1:cc + 1])

            # one-tile software pipeline: pass-1 of tile t+1 is emitted before
            # pass-2 of tile t so the PE never waits on the T evictions
            prev = None
            for bt in range(NBT):
                tsb = pass1(bt)
                if prev is not None:
                    pass2(bt - 1, prev)
                prev = tsb
            pass2(NBT - 1, prev)

    nc.compile()
    return nc


def kernel(x: np.ndarray, twiddle: np.ndarray, bias: np.ndarray) -> np.ndarray:
    global _last_exec_time_ns, _nc_cache

    bl_pack, d_pack = _host_weights(twiddle)
    bl_host = np.ascontiguousarray(bl_pack.astype(ml_dtypes.bfloat16))
    d_host = np.ascontiguousarray(d_pack.astype(ml_dtypes.bfloat16))
    bb_host = _stored_bias(np.asarray(bias))

    x = np.ascontiguousarray(x, dtype=np.float32)
    xb = x.astype(ml_dtypes.bfloat16)
    # [cores, NBT, 128 part, 8 w, BT] with tile g contiguous in HBM
    xtb_all = np.ascontiguousarray(
        xb.reshape(N_CORES, NBT, BT, 8, 128).transpose(0, 1, 4, 3, 2)
    )

    if _nc_cache is None:
        _nc_cache = _build_nc()
    nc = _nc_cache

    in_maps = [
        {"xtb": xtb_all[i], "bl": bl_host, "dd": d_host, "bb": bb_host}
        for i in range(N_CORES)
    ]

    trace = bool(int(os.environ.get("BUTTERFLY_TRACE", "0")))
    res = run_bass_kernel_spmd(
        nc,
        in_maps,
        core_ids=list(range(N_CORES)),
        trace=trace,
    )
    _last_exec_time_ns = res.exec_time_ns

    outs = []
    for i in range(N_CORES):
        o = np.asarray(res.results[i]["out"])  # [NBT, 128, 4, N] bf16
        o = o.astype(np.float32).transpose(0, 2, 1, 3).reshape(BC, N)
        outs.append(o)
    return np.concatenate(outs, axis=0)



# revision 6
# speedup vs baseline: 1.0797x; 1.0797x over previous
"""v5: two-pass butterfly, weights-stationary pass 2, feature-major output.

Factor B = Bh @ Bl:
  Bl = stages 0..6  — block-diagonal over 8 contiguous 128-position blocks.
  Bh = stages 7..9  — mixes w = pos//128 across the 8 blocks, elementwise in
                      r = pos % 128.

Pass 1 (per 512-batch tile bt): T tiles in interleaved partition order.
  T[m][h] [128, 512]: partition p' = 32*wl + rl  <->  pos (32m + rl) + 128*(4h+wl)
  built by col-tiled quads (M=32, tile_position) with lhsT = Bl^T block slice,
  rhs = x block [128, 512]; one psum [128, 2, 512] per (bt, m), evicted to a
  resident T_big sbuf tile (bf16), alternating Scalar/Vector.

Pass 2 (per bt-pair p): out^T in feature-major, D stationary.
  psum[q, b] = sum_h D[m][h][:, qh-slice]^T @ T[m][h][bt]   (q = 32*wo' + rl)
  One DVE tensor_scalar_add per psum tile fuses the bias (per-partition
  column) and writes bf16 to the osb staging tile; stores ride the scalar
  engine's HWDGE ring (separate FIFO from the sync-engine loads).
  Host transposes the feature-major output back (free).

Extras: ~10 warm-up matmuls on a zeroed tile at t=0 keep the PE HAM
clock-gate open during the DMA lead-in; all loads are 8KB-per-partition
contiguous descriptors issued upfront on the sync ring.
"""

import os
import sys
import numpy as np

for _p in ("/opt/trn_rl_repo", os.path.expanduser("~/.axon_site/_ro/trn_rl_repo")):
    if os.path.isdir(_p) and _p not in sys.path:
        sys.path.insert(0, _p)

import concourse.bass as bass
import concourse.bacc as bacc
import concourse.mybir as mybir
from concourse import tile
from concourse.bass_utils import run_bass_kernel_spmd

import ml_dtypes

N_CORES = 8
BATCH = 32768
N = 1024
LOG_N = 10
BC = BATCH // N_CORES   # 4096 rows per core
BT = 512                # batch tile (pass 1)
NBT = BC // BT          # 8

_last_exec_time_ns = None
_nc_cache = None


def _apply_stages(m: np.ndarray, twiddle: np.ndarray, idxs) -> np.ndarray:
    """Apply butterfly stages `idxs` to the rows of m (batch of vectors)."""
    n = N
    for idx in idxs:
        s = 1 << idx
        g = n // (2 * s)
        t = twiddle[0, 0, idx].astype(np.float64).reshape(g, s, 2, 2)
        xr = m.reshape(-1, g, 2, s)
        m = np.einsum("grij,bgjr->bgir", t, xr).reshape(-1, n)
    return m


def _host_weights(twiddle: np.ndarray):
    eye = np.eye(N, dtype=np.float64)
    blt = _apply_stages(eye, twiddle, range(7))        # BlT[k, p] = Bl[p, k]
    bht = _apply_stages(eye, twiddle, range(7, 10))    # BhT[k, p] = Bh[p, k]

    # pass-1 lhsT: bl_pack[k, w, m, r32] = Bl[128w + 32m + r32, 128w + k]
    bl_pack = np.zeros((128, 8, 4, 32), dtype=np.float64)
    for w in range(8):
        blk = blt[128 * w:128 * (w + 1), 128 * w:128 * (w + 1)]  # [k, r]
        bl_pack[:, w] = blk.reshape(128, 4, 32)

    # pass-2 stationary operand: d_pack[p', m, h, q]
    #   p' = 32*wl + rl_in  -> pos_in  = 32m + rl_in + 128*(4h + wl)
    #   q  = 32*w_out + rl_out -> pos_out = 32m + rl_out + 128*w_out
    # value = BhT[pos_in, pos_out] = Bh[pos_out, pos_in]
    wl = np.arange(4)[:, None]          # [4, 1]
    rl = np.arange(32)[None, :]         # [1, 32]
    wo = np.arange(8)[:, None]
    d_pack = np.zeros((128, 4, 2, 256), dtype=np.float64)
    for m in range(4):
        for h in range(2):
            pos_in = (32 * m + rl + 128 * (4 * h + wl))        # [4, 32]
            pos_out = (32 * m + rl + 128 * wo)                 # [8, 32]
            # nonzero only when rl_in == rl_out
            sub = bht[np.ix_(pos_in.ravel(), pos_out.ravel())]  # [128, 256]
            mask = (rl.ravel()[None, :].repeat(4, 0).ravel()[:, None]
                    == rl.ravel()[None, :].repeat(8, 0).ravel()[None, :])
            d_pack[:, m, h, :] = np.where(mask, sub, 0.0)

    return bl_pack, d_pack


def _bias_cols(bias: np.ndarray) -> np.ndarray:
    # bias_col[p = 32*wo' + rl, g = 2m + qh] = bias[128*(4qh + wo') + 32m + rl]
    out = np.zeros((128, 8), dtype=np.float32)
    wo = np.arange(4)[:, None]
    rl = np.arange(32)[None, :]
    for m in range(4):
        for qh in range(2):
            pos = 128 * (4 * qh + wo) + 32 * m + rl   # [4, 32]
            out[:, 2 * m + qh] = bias[pos.ravel()].astype(np.float32)
    return np.ascontiguousarray(out)


def _build_nc():
    nc = bacc.Bacc("TRN2", target_bir_lowering=False)
    xtb = nc.dram_tensor("xtb", [NBT, 128, 8, BT], mybir.dt.bfloat16, kind="ExternalInput")
    bl = nc.dram_tensor("bl", [128, 8, 4, 32], mybir.dt.bfloat16, kind="ExternalInput")
    dd = nc.dram_tensor("dd", [128, 4, 2, 256], mybir.dt.bfloat16, kind="ExternalInput")
    bb = nc.dram_tensor("bb", [128, 8], mybir.dt.float32, kind="ExternalInput")
    out = nc.dram_tensor("out", [8, 128, 4, 2, BT], mybir.dt.bfloat16,
                         kind="ExternalOutput")

    with tile.TileContext(nc) as tc:
        with (
            tc.tile_pool(name="const", bufs=1) as cpool,
            tc.tile_pool(name="ps1", bufs=2, space="PSUM") as ps1_pool,
            tc.tile_pool(name="ps2", bufs=2, space="PSUM") as ps2_pool,
        ):
            # warm-up source (zeros) — matmuls on it keep the PE busy so the
            # HAM clock-gate opens while the first x tiles stream in
            warm = cpool.tile([128, 512], mybir.dt.bfloat16)
            nc.gpsimd.memset(warm[:], 0)

            # loads: everything upfront on the sync HWDGE ring, in the order
            # compute needs it; all are per-partition-contiguous descriptors
            bls = cpool.tile([128, 8, 4, 32], mybir.dt.bfloat16)
            nc.sync.dma_start(out=bls[:], in_=bl[:])

            xall = cpool.tile([128, NBT, 8, BT], mybir.dt.bfloat16)
            nc.sync.dma_start(out=xall[:, 0, 0:4], in_=xtb[0][:, 0:4])
            nc.sync.dma_start(out=xall[:, 0, 4:8], in_=xtb[0][:, 4:8])

            dds = cpool.tile([128, 4, 2, 256], mybir.dt.bfloat16)
            nc.sync.dma_start(out=dds[:], in_=dd[:])
            bbt = cpool.tile([128, 8], mybir.dt.float32)
            nc.sync.dma_start(out=bbt[:], in_=bb[:])

            for g in range(1, NBT):
                nc.sync.dma_start(out=xall[:, g], in_=xtb[g])

            # warm-up matmuls (results discarded); the psum tile shares the
            # pass-1 pool slots (tag="ps") so no extra PSUM is reserved
            wps = ps1_pool.tile([128, 2, 512], mybir.dt.float32, tag="ps")
            for _ in range(10):
                nc.tensor.matmul(wps[:, 0, :], warm[:, 0:128], warm[:],
                                 start=True, stop=True)

            # resident intermediate: T_big[p', m, h, bt, b]
            t_big = cpool.tile([128, 4, 2, NBT, BT], mybir.dt.bfloat16)
            # output staging, double-buffered over pair parity
            osb = cpool.tile([128, 8, 2, 2, BT], mybir.dt.bfloat16)

            def pass1_bt(bt):
                for m in range(4):
                    ps = ps1_pool.tile([128, 2, 512], mybir.dt.float32)
                    for h in range(2):
                        for wl in range(4):
                            w = 4 * h + wl
                            nc.tensor.matmul(
                                ps[32 * wl:32 * (wl + 1), h, :],
                                bls[:, w, m, :],
                                xall[:, bt, w, :],
                                start=True,
                                stop=True,
                                tile_position=(0, 32 * wl),
                            )
                    if (bt * 4 + m) % 2 == 0:
                        nc.scalar.copy(out=t_big[:, m, :, bt, :], in_=ps[:])
                    else:
                        nc.vector.tensor_copy(out=t_big[:, m, :, bt, :], in_=ps[:])

            def pass2_pair(p):
                for m in range(4):
                    for qh in range(2):
                        g = 2 * m + qh
                        ps = ps2_pool.tile([128, 2, 512], mybir.dt.float32)
                        for c in range(2):
                            bt = 2 * p + c
                            for h in range(2):
                                nc.tensor.matmul(
                                    ps[:, c, :],
                                    dds[:, m, h, 128 * qh:128 * (qh + 1)],
                                    t_big[:, m, h, bt, :],
                                    start=(h == 0),
                                    stop=(h == 1),
                                )
                        nc.vector.tensor_scalar_add(
                            osb[:, g, p % 2],
                            ps[:],
                            bbt[:, g:g + 1],
                        )
                        # store this pair's columns; scalar HWDGE ring so the
                        # output stream doesn't queue behind the loads
                        nc.scalar.dma_start(
                            out=out[g][:, p],
                            in_=osb[:, g, p % 2],
                        )

            # interleave pass1 bt-pairs with pass2 sweeps so the in-order PE
            # program fills DMA-wait gaps with useful matmuls
            for p in range(4):
                pass1_bt(2 * p)
                pass1_bt(2 * p + 1)
                pass2_pair(p)

    nc.compile()
    return nc


def kernel(x: np.ndarray, twiddle: np.ndarray, bias: np.ndarray) -> np.ndarray:
    global _last_exec_time_ns, _nc_cache

    bl_pack, d_pack = _host_weights(twiddle)
    bl_host = np.ascontiguousarray(bl_pack.astype(ml_dtypes.bfloat16))
    d_host = np.ascontiguousarray(d_pack.astype(ml_dtypes.bfloat16))
    bb_host = _bias_cols(np.asarray(bias))

    x = np.ascontiguousarray(x, dtype=np.float32)
    xb = x.astype(ml_dtypes.bfloat16)
    # [cores, NBT, 128 part, 8 w, BT] with tile g contiguous in HBM
    xtb_all = np.ascontiguousarray(
        xb.reshape(N_CORES, NBT, BT, 8, 128).transpose(0, 1, 4, 3, 2)
    )

    if _nc_cache is None:
        _nc_cache = _build_nc()
    nc = _nc_cache

    in_maps = [
        {"xtb": xtb_all[i], "bl": bl_host, "dd": d_host, "bb": bb_host}
        for i in range(N_CORES)
    ]

    trace = bool(int(os.environ.get("BUTTERFLY_TRACE", "0")))
    res = run_bass_kernel_spmd(
        nc,
        in_maps,
        core_ids=list(range(N_CORES)),
        trace=trace,
    )
    _last_exec_time_ns = res.exec_time_ns

    outs = []
    for i in range(N_CORES):
        o = np.asarray(res.results[i]["out"])  # [8 g, 128 q, 4096 b] bf16
        # g = 2m + qh, q = 32*wo' + rl; pos = 128*(4qh + wo') + 32m + rl
        o = o.astype(np.float32).reshape(4, 2, 4, 32, BC)
        o = o.transpose(4, 1, 2, 0, 3).reshape(BC, N)
        outs.append(o)
    return np.concatenate(outs, axis=0)


# revision 8
# speedup vs baseline: 1.1168x; 1.0343x over previous
"""v5: two-pass butterfly, weights-stationary pass 2, feature-major output.

Factor B = Bh @ Bl:
  Bl = stages 0..6  — block-diagonal over 8 contiguous 128-position blocks.
  Bh = stages 7..9  — mixes w = pos//128 across the 8 blocks, elementwise in
                      r = pos % 128.

Pass 1 (per 512-batch tile bt): T tiles in interleaved partition order.
  T[m][h] [128, 512]: partition p' = 32*wl + rl  <->  pos (32m + rl) + 128*(4h+wl)
  built by col-tiled quads (M=32, tile_position) with lhsT = Bl^T block slice,
  rhs = x block [128, 512]; one psum [128, 2, 512] per (bt, m), evicted to a
  resident T_big sbuf tile (bf16), alternating Scalar/Vector.

Pass 2 (per bt-pair p): out^T in feature-major, D stationary.
  psum[q, b] = sum_h D[m][h][:, qh-slice]^T @ T[m][h][bt]   (q = 32*wo' + rl)
  One DVE tensor_scalar_add per psum tile fuses the bias (per-partition
  column) and writes bf16 to the osb staging tile; stores ride the scalar
  engine's HWDGE ring (separate FIFO from the sync-engine loads).
  Host transposes the feature-major output back (free).

Extras: ~10 warm-up matmuls on a zeroed tile at t=0 keep the PE HAM
clock-gate open during the DMA lead-in; all loads are 8KB-per-partition
contiguous descriptors issued upfront on the sync ring.
"""

import os
import sys
import numpy as np

for _p in ("/opt/trn_rl_repo", os.path.expanduser("~/.axon_site/_ro/trn_rl_repo")):
    if os.path.isdir(_p) and _p not in sys.path:
        sys.path.insert(0, _p)

import concourse.bass as bass
import concourse.bacc as bacc
import concourse.mybir as mybir
from concourse import tile
from concourse.bass_utils import run_bass_kernel_spmd

import ml_dtypes

N_CORES = 8
BATCH = 32768
N = 1024
LOG_N = 10
BC = BATCH // N_CORES   # 4096 rows per core
BT = 512                # batch tile (pass 1)
NBT = BC // BT          # 8

_last_exec_time_ns = None
_nc_cache = None


def _apply_stages(m: np.ndarray, twiddle: np.ndarray, idxs) -> np.ndarray:
    """Apply butterfly stages `idxs` to the rows of m (batch of vectors)."""
    n = N
    for idx in idxs:
        s = 1 << idx
        g = n // (2 * s)
        t = twiddle[0, 0, idx].astype(np.float64).reshape(g, s, 2, 2)
        xr = m.reshape(-1, g, 2, s)
        m = np.einsum("grij,bgjr->bgir", t, xr).reshape(-1, n)
    return m


def _host_weights(twiddle: np.ndarray):
    eye = np.eye(N, dtype=np.float64)
    blt = _apply_stages(eye, twiddle, range(7))        # BlT[k, p] = Bl[p, k]
    bht = _apply_stages(eye, twiddle, range(7, 10))    # BhT[k, p] = Bh[p, k]

    # pass-1 lhsT: bl_pack[k, w, m, r32] = Bl[128w + 32m + r32, 128w + k]
    bl_pack = np.zeros((128, 8, 4, 32), dtype=np.float64)
    for w in range(8):
        blk = blt[128 * w:128 * (w + 1), 128 * w:128 * (w + 1)]  # [k, r]
        bl_pack[:, w] = blk.reshape(128, 4, 32)

    # pass-2 stationary operand: d_pack[p', m, h, q]
    #   p' = 32*wl + rl_in  -> pos_in  = 32m + rl_in + 128*(4h + wl)
    #   q  = 32*w_out + rl_out -> pos_out = 32m + rl_out + 128*w_out
    # value = BhT[pos_in, pos_out] = Bh[pos_out, pos_in]
    wl = np.arange(4)[:, None]          # [4, 1]
    rl = np.arange(32)[None, :]         # [1, 32]
    wo = np.arange(8)[:, None]
    d_pack = np.zeros((128, 4, 2, 256), dtype=np.float64)
    for m in range(4):
        for h in range(2):
            pos_in = (32 * m + rl + 128 * (4 * h + wl))        # [4, 32]
            pos_out = (32 * m + rl + 128 * wo)                 # [8, 32]
            # nonzero only when rl_in == rl_out
            sub = bht[np.ix_(pos_in.ravel(), pos_out.ravel())]  # [128, 256]
            mask = (rl.ravel()[None, :].repeat(4, 0).ravel()[:, None]
                    == rl.ravel()[None, :].repeat(8, 0).ravel()[None, :])
            d_pack[:, m, h, :] = np.where(mask, sub, 0.0)

    return bl_pack, d_pack


def _bias_cols(bias: np.ndarray) -> np.ndarray:
    # bias_col[p = 32*wo' + rl, g = 2m + qh] = bias[128*(4qh + wo') + 32m + rl]
    out = np.zeros((128, 8), dtype=np.float32)
    wo = np.arange(4)[:, None]
    rl = np.arange(32)[None, :]
    for m in range(4):
        for qh in range(2):
            pos = 128 * (4 * qh + wo) + 32 * m + rl   # [4, 32]
            out[:, 2 * m + qh] = bias[pos.ravel()].astype(np.float32)
    return np.ascontiguousarray(out)


def _build_nc():
    nc = bacc.Bacc("TRN2", target_bir_lowering=False)
    xtb = nc.dram_tensor("xtb", [NBT, 128, 8, BT], mybir.dt.bfloat16, kind="ExternalInput")
    bl = nc.dram_tensor("bl", [128, 8, 4, 32], mybir.dt.bfloat16, kind="ExternalInput")
    dd = nc.dram_tensor("dd", [128, 4, 2, 256], mybir.dt.bfloat16, kind="ExternalInput")
    bb = nc.dram_tensor("bb", [128, 8], mybir.dt.float32, kind="ExternalInput")
    out = nc.dram_tensor("out", [8, 128, 4, 2, BT], mybir.dt.bfloat16,
                         kind="ExternalOutput")

    with tile.TileContext(nc) as tc:
        with (
            tc.tile_pool(name="const", bufs=1) as cpool,
            tc.tile_pool(name="ps1", bufs=2, space="PSUM") as ps1_pool,
            tc.tile_pool(name="ps2", bufs=2, space="PSUM") as ps2_pool,
        ):
            # warm-up source (zeros) — matmuls on it keep the PE busy so the
            # HAM clock-gate opens while the first x tiles stream in
            warm = cpool.tile([128, 512], mybir.dt.bfloat16)
            nc.gpsimd.memset(warm[:], 0)

            # loads: everything upfront on the sync HWDGE ring, in the order
            # compute needs it; all are per-partition-contiguous descriptors
            bls = cpool.tile([128, 8, 4, 32], mybir.dt.bfloat16)
            nc.sync.dma_start(out=bls[:], in_=bl[:])

            xall = cpool.tile([128, NBT, 8, BT], mybir.dt.bfloat16)
            nc.sync.dma_start(out=xall[:, 0, 0:4], in_=xtb[0][:, 0:4])
            nc.sync.dma_start(out=xall[:, 0, 4:8], in_=xtb[0][:, 4:8])

            dds = cpool.tile([128, 4, 2, 256], mybir.dt.bfloat16)
            nc.sync.dma_start(out=dds[:], in_=dd[:])
            bbt = cpool.tile([128, 8], mybir.dt.float32)
            nc.sync.dma_start(out=bbt[:], in_=bb[:])

            for g in range(1, NBT):
                nc.sync.dma_start(out=xall[:, g], in_=xtb[g])

            # warm-up matmuls (results discarded); the psum tile shares the
            # pass-1 pool slots (tag="ps") so no extra PSUM is reserved
            wps = ps1_pool.tile([128, 2, 512], mybir.dt.float32, tag="ps")
            for _ in range(14):
                nc.tensor.matmul(wps[:, 0, :], warm[:, 0:128], warm[:],
                                 start=True, stop=True)

            # resident intermediate: T_big[p', m, h, bt, b]
            t_big = cpool.tile([128, 4, 2, NBT, BT], mybir.dt.bfloat16)
            # output staging, double-buffered over pair parity
            osb = cpool.tile([128, 8, 2, 2, BT], mybir.dt.bfloat16)

            # evictions are the co-bottleneck: 64 psum->sbuf units split
            # between ScalarE (~1.11us/unit) and VectorE (~1.25us/unit);
            # Bresenham split 34:30 balances their busy time
            evict_state = [0, 0]  # units seen, units given to ACT

            def use_act():
                evict_state[0] += 1
                target = (evict_state[0] * 34 + 32) // 64
                if evict_state[1] < target:
                    evict_state[1] += 1
                    return True
                return False

            def pass1_bt(bt):
                for m in range(4):
                    ps = ps1_pool.tile([128, 2, 512], mybir.dt.float32)
                    for h in range(2):
                        for wl in range(4):
                            w = 4 * h + wl
                            nc.tensor.matmul(
                                ps[32 * wl:32 * (wl + 1), h, :],
                                bls[:, w, m, :],
                                xall[:, bt, w, :],
                                start=True,
                                stop=True,
                                tile_position=(0, 32 * wl),
                            )
                    if use_act():
                        nc.scalar.copy(out=t_big[:, m, :, bt, :], in_=ps[:])
                    else:
                        nc.vector.tensor_copy(out=t_big[:, m, :, bt, :], in_=ps[:])

            def pass2_pair(p):
                for m in range(4):
                    for qh in range(2):
                        g = 2 * m + qh
                        ps = ps2_pool.tile([128, 2, 512], mybir.dt.float32)
                        for c in range(2):
                            bt = 2 * p + c
                            for h in range(2):
                                nc.tensor.matmul(
                                    ps[:, c, :],
                                    dds[:, m, h, 128 * qh:128 * (qh + 1)],
                                    t_big[:, m, h, bt, :],
                                    start=(h == 0),
                                    stop=(h == 1),
                                )
                        if use_act():
                            nc.scalar.activation(
                                osb[:, g, p % 2],
                                ps[:],
                                mybir.ActivationFunctionType.Identity,
                                bias=bbt[:, g:g + 1],
                            )
                        else:
                            nc.vector.tensor_scalar_add(
                                osb[:, g, p % 2],
                                ps[:],
                                bbt[:, g:g + 1],
                            )
                        # stores ride the (otherwise idle) GPSIMD SWDGE queue
                        # so neither compute engine pays descriptor-gen time
                        nc.gpsimd.dma_start(
                            out=out[g][:, p],
                            in_=osb[:, g, p % 2],
                        )

            # interleave pass1 bt-pairs with pass2 sweeps so the in-order PE
            # program fills DMA-wait gaps with useful matmuls
            for p in range(4):
                pass1_bt(2 * p)
                pass1_bt(2 * p + 1)
                pass2_pair(p)

    nc.compile()
    return nc


def kernel(x: np.ndarray, twiddle: np.ndarray, bias: np.ndarray) -> np.ndarray:
    global _last_exec_time_ns, _nc_cache

    bl_pack, d_pack = _host_weights(twiddle)
    bl_host = np.ascontiguousarray(bl_pack.astype(ml_dtypes.bfloat16))
    d_host = np.ascontiguousarray(d_pack.astype(ml_dtypes.bfloat16))
    bb_host = _bias_cols(np.asarray(bias))

    x = np.ascontiguousarray(x, dtype=np.float32)
    xb = x.astype(ml_dtypes.bfloat16)
    # [cores, NBT, 128 part, 8 w, BT] with tile g contiguous in HBM
    xtb_all = np.ascontiguousarray(
        xb.reshape(N_CORES, NBT, BT, 8, 128).transpose(0, 1, 4, 3, 2)
    )

    if _nc_cache is None:
        _nc_cache = _build_nc()
    nc = _nc_cache

    in_maps = [
        {"xtb": xtb_all[i], "bl": bl_host, "dd": d_host, "bb": bb_host}
        for i in range(N_CORES)
    ]

    trace = bool(int(os.environ.get("BUTTERFLY_TRACE", "0")))
    res = run_bass_kernel_spmd(
        nc,
        in_maps,
        core_ids=list(range(N_CORES)),
        trace=trace,
    )
    _last_exec_time_ns = res.exec_time_ns

    outs = []
    for i in range(N_CORES):
        o = np.asarray(res.results[i]["out"])  # [8 g, 128 q, 4096 b] bf16
        # g = 2m + qh, q = 32*wo' + rl; pos = 128*(4qh + wo') + 32m + rl
        o = o.astype(np.float32).reshape(4, 2, 4, 32, BC)
        o = o.transpose(4, 1, 2, 0, 3).reshape(BC, N)
        outs.append(o)
    return np.concatenate(outs, axis=0)


# revision 12
# speedup vs baseline: 1.3779x; 1.2339x over previous
"""v5: two-pass butterfly, weights-stationary pass 2, feature-major output.

Factor B = Bh @ Bl:
  Bl = stages 0..6  — block-diagonal over 8 contiguous 128-position blocks.
  Bh = stages 7..9  — mixes w = pos//128 across the 8 blocks, elementwise in
                      r = pos % 128.

Pass 1 (per 512-batch tile bt): T tiles in interleaved partition order.
  T[m][h] [128, 512]: partition p' = 32*wl + rl  <->  pos (32m + rl) + 128*(4h+wl)
  built by col-tiled quads (M=32, tile_position) with lhsT = Bl^T block slice,
  rhs = x block [128, 512]; one psum [128, 2, 512] per (bt, m), evicted to a
  resident T_big sbuf tile (bf16), alternating Scalar/Vector.

Pass 2 (per bt-pair p): out^T in feature-major, D stationary.
  psum[q, b] = sum_h D[m][h][:, qh-slice]^T @ T[m][h][bt]   (q = 32*wo' + rl)
  One DVE tensor_scalar_add per psum tile fuses the bias (per-partition
  column) and writes bf16 to the osb staging tile; stores ride the scalar
  engine's HWDGE ring (separate FIFO from the sync-engine loads).
  Host transposes the feature-major output back (free).

Extras: ~10 warm-up matmuls on a zeroed tile at t=0 keep the PE HAM
clock-gate open during the DMA lead-in; all loads are 8KB-per-partition
contiguous descriptors issued upfront on the sync ring.
"""

import os
import sys
import numpy as np

for _p in ("/opt/trn_rl_repo", os.path.expanduser("~/.axon_site/_ro/trn_rl_repo")):
    if os.path.isdir(_p) and _p not in sys.path:
        sys.path.insert(0, _p)

import concourse.bass as bass
import concourse.bacc as bacc
import concourse.mybir as mybir
from concourse import tile
from concourse.bass_utils import run_bass_kernel_spmd

import ml_dtypes

N_CORES = 8
BATCH = 32768
N = 1024
LOG_N = 10
BC = BATCH // N_CORES   # 4096 rows per core
BT = 512                # batch tile (pass 1)
NBT = BC // BT          # 8

_last_exec_time_ns = None
_nc_cache = None


def _apply_stages(m: np.ndarray, twiddle: np.ndarray, idxs) -> np.ndarray:
    """Apply butterfly stages `idxs` to the rows of m (batch of vectors)."""
    n = N
    for idx in idxs:
        s = 1 << idx
        g = n // (2 * s)
        t = twiddle[0, 0, idx].astype(np.float64).reshape(g, s, 2, 2)
        xr = m.reshape(-1, g, 2, s)
        m = np.einsum("grij,bgjr->bgir", t, xr).reshape(-1, n)
    return m


def _host_weights(twiddle: np.ndarray):
    eye = np.eye(N, dtype=np.float64)
    blt = _apply_stages(eye, twiddle, range(7))        # BlT[k, p] = Bl[p, k]
    bht = _apply_stages(eye, twiddle, range(7, 10))    # BhT[k, p] = Bh[p, k]

    # pass-1 lhsT: bl_pack[k, w, m, r32] = Bl[128w + 32m + r32, 128w + k]
    bl_pack = np.zeros((128, 8, 4, 32), dtype=np.float64)
    for w in range(8):
        blk = blt[128 * w:128 * (w + 1), 128 * w:128 * (w + 1)]  # [k, r]
        bl_pack[:, w] = blk.reshape(128, 4, 32)

    # pass-2 stationary operand: d_pack[p', m, h, q]
    #   p' = 32*wl + rl_in  -> pos_in  = 32m + rl_in + 128*(4h + wl)
    #   q  = 32*w_out + rl_out -> pos_out = 32m + rl_out + 128*w_out
    # value = BhT[pos_in, pos_out] = Bh[pos_out, pos_in]
    wl = np.arange(4)[:, None]          # [4, 1]
    rl = np.arange(32)[None, :]         # [1, 32]
    wo = np.arange(8)[:, None]
    d_pack = np.zeros((128, 4, 2, 256), dtype=np.float64)
    for m in range(4):
        for h in range(2):
            pos_in = (32 * m + rl + 128 * (4 * h + wl))        # [4, 32]
            pos_out = (32 * m + rl + 128 * wo)                 # [8, 32]
            # nonzero only when rl_in == rl_out
            sub = bht[np.ix_(pos_in.ravel(), pos_out.ravel())]  # [128, 256]
            mask = (rl.ravel()[None, :].repeat(4, 0).ravel()[:, None]
                    == rl.ravel()[None, :].repeat(8, 0).ravel()[None, :])
            d_pack[:, m, h, :] = np.where(mask, sub, 0.0)

    return bl_pack, d_pack


def _bias_cols(bias: np.ndarray) -> np.ndarray:
    # bias_col[p = 32*wo' + rl, g = 2m + qh] = bias[128*(4qh + wo') + 32m + rl]
    out = np.zeros((128, 8), dtype=np.float32)
    wo = np.arange(4)[:, None]
    rl = np.arange(32)[None, :]
    for m in range(4):
        for qh in range(2):
            pos = 128 * (4 * qh + wo) + 32 * m + rl   # [4, 32]
            out[:, 2 * m + qh] = bias[pos.ravel()].astype(np.float32)
    return np.ascontiguousarray(out)


def _build_nc():
    nc = bacc.Bacc("TRN2", target_bir_lowering=False)
    xtb = nc.dram_tensor("xtb", [NBT, 128, 8, BT], mybir.dt.bfloat16, kind="ExternalInput")
    bl = nc.dram_tensor("bl", [128, 8, 4, 32], mybir.dt.bfloat16, kind="ExternalInput")
    dd = nc.dram_tensor("dd", [128, 4, 2, 256], mybir.dt.bfloat16, kind="ExternalInput")
    bb = nc.dram_tensor("bb", [128, 8], mybir.dt.float32, kind="ExternalInput")
    out = nc.dram_tensor("out", [8, 128, 4, 2, BT], mybir.dt.bfloat16,
                         kind="ExternalOutput")

    with tile.TileContext(nc) as tc:
        with (
            tc.tile_pool(name="const", bufs=1) as cpool,
            # one shared psum pool: pass-1 units, pass-2 sweeps and warm-up
            # all use the same [128, 2, 512] f32 shape under one tag, so the
            # 4 bufs (8 banks) give 4-deep pipelining to whichever phase is
            # active instead of 2+2 split statically
            tc.tile_pool(name="psp", bufs=4, space="PSUM") as ps_pool,
        ):
            # warm-up source (zeros) — matmuls on it keep the PE busy so the
            # HAM clock-gate opens while the first x tiles stream in
            warm = cpool.tile([128, 512], mybir.dt.bfloat16)
            nc.gpsimd.memset(warm[:], 0)

            # loads: everything upfront on the sync HWDGE ring, in the order
            # compute needs it; all are per-partition-contiguous descriptors
            bls = cpool.tile([128, 8, 4, 32], mybir.dt.bfloat16)
            nc.sync.dma_start(out=bls[:], in_=bl[:])

            xall = cpool.tile([128, NBT, 8, BT], mybir.dt.bfloat16)
            nc.sync.dma_start(out=xall[:, 0, 0:4], in_=xtb[0][:, 0:4])
            nc.sync.dma_start(out=xall[:, 0, 4:8], in_=xtb[0][:, 4:8])

            dds = cpool.tile([128, 4, 2, 256], mybir.dt.bfloat16)
            nc.sync.dma_start(out=dds[:], in_=dd[:])
            bbt = cpool.tile([128, 8], mybir.dt.float32)
            nc.sync.dma_start(out=bbt[:], in_=bb[:])

            for g in range(1, NBT):
                nc.sync.dma_start(out=xall[:, g], in_=xtb[g])

            # warm-up matmuls (results discarded) bridge the PE from t=0 to
            # the first x tile so the HAM clock-gate opens and stays open
            wps = ps_pool.tile([128, 2, 512], mybir.dt.float32, tag="ps")
            for _ in range(20):
                nc.tensor.matmul(wps[:, 0, :], warm[:, 0:128], warm[:],
                                 start=True, stop=True)

            # resident intermediate: T_big[p', m, h, bt, b]
            t_big = cpool.tile([128, 4, 2, NBT, BT], mybir.dt.bfloat16)
            # output staging, double-buffered over pair parity
            osb = cpool.tile([128, 8, 2, 2, BT], mybir.dt.bfloat16)

            # evictions are the co-bottleneck: 64 psum->sbuf units split
            # between ScalarE (~1.11us/unit) and VectorE (~1.25us/unit);
            # Bresenham split 34:30 balances their busy time
            evict_state = [0, 0]  # units seen, units given to ACT

            def use_act():
                evict_state[0] += 1
                target = (evict_state[0] * 34 + 32) // 64
                if evict_state[1] < target:
                    evict_state[1] += 1
                    return True
                return False

            def pass1_bt(bt):
                for m in range(4):
                    ps = ps_pool.tile([128, 2, 512], mybir.dt.float32, tag="ps")
                    for h in range(2):
                        for wl in range(4):
                            w = 4 * h + wl
                            nc.tensor.matmul(
                                ps[32 * wl:32 * (wl + 1), h, :],
                                bls[:, w, m, :],
                                xall[:, bt, w, :],
                                start=True,
                                stop=True,
                                tile_position=(0, 32 * wl),
                            )
                    if use_act():
                        nc.scalar.copy(out=t_big[:, m, :, bt, :], in_=ps[:])
                    else:
                        nc.vector.tensor_copy(out=t_big[:, m, :, bt, :], in_=ps[:])

            def pass2_pair(p):
                for m in range(4):
                    for qh in range(2):
                        g = 2 * m + qh
                        ps = ps_pool.tile([128, 2, 512], mybir.dt.float32, tag="ps")
                        for c in range(2):
                            bt = 2 * p + c
                            for h in range(2):
                                nc.tensor.matmul(
                                    ps[:, c, :],
                                    dds[:, m, h, 128 * qh:128 * (qh + 1)],
                                    t_big[:, m, h, bt, :],
                                    start=(h == 0),
                                    stop=(h == 1),
                                )
                        if use_act():
                            nc.scalar.activation(
                                osb[:, g, p % 2],
                                ps[:],
                                mybir.ActivationFunctionType.Identity,
                                bias=bbt[:, g:g + 1],
                            )
                        else:
                            nc.vector.tensor_scalar_add(
                                osb[:, g, p % 2],
                                ps[:],
                                bbt[:, g:g + 1],
                            )
                        # stores ride the (otherwise idle) GPSIMD SWDGE queue
                        # so neither compute engine pays descriptor-gen time
                        nc.gpsimd.dma_start(
                            out=out[g][:, p],
                            in_=osb[:, g, p % 2],
                        )

            # interleave pass1 bt-pairs with pass2 sweeps so the in-order PE
            # program fills DMA-wait gaps with useful matmuls
            for p in range(4):
                pass1_bt(2 * p)
                pass1_bt(2 * p + 1)
                pass2_pair(p)

    nc.compile()
    return nc


def kernel(x: np.ndarray, twiddle: np.ndarray, bias: np.ndarray) -> np.ndarray:
    global _last_exec_time_ns, _nc_cache

    bl_pack, d_pack = _host_weights(twiddle)
    bl_host = np.ascontiguousarray(bl_pack.astype(ml_dtypes.bfloat16))
    d_host = np.ascontiguousarray(d_pack.astype(ml_dtypes.bfloat16))
    bb_host = _bias_cols(np.asarray(bias))

    x = np.ascontiguousarray(x, dtype=np.float32)
    xb = x.astype(ml_dtypes.bfloat16)
    # [cores, NBT, 128 part, 8 w, BT] with tile g contiguous in HBM
    xtb_all = np.ascontiguousarray(
        xb.reshape(N_CORES, NBT, BT, 8, 128).transpose(0, 1, 4, 3, 2)
    )

    if _nc_cache is None:
        _nc_cache = _build_nc()
    nc = _nc_cache

    in_maps = [
        {"xtb": xtb_all[i], "bl": bl_host, "dd": d_host, "bb": bb_host}
        for i in range(N_CORES)
    ]

    trace = bool(int(os.environ.get("BUTTERFLY_TRACE", "0")))
    res = run_bass_kernel_spmd(
        nc,
        in_maps,
        core_ids=list(range(N_CORES)),
        trace=trace,
    )
    _last_exec_time_ns = res.exec_time_ns

    outs = []
    for i in range(N_CORES):
        o = np.asarray(res.results[i]["out"])  # [8 g, 128 q, 4096 b] bf16
        # g = 2m + qh, q = 32*wo' + rl; pos = 128*(4qh + wo') + 32m + rl
        o = o.astype(np.float32).reshape(4, 2, 4, 32, BC)
        o = o.transpose(4, 1, 2, 0, 3).reshape(BC, N)
        outs.append(o)
    return np.concatenate(outs, axis=0)


# revision 14
# speedup vs baseline: 1.4423x; 1.0467x over previous
"""v5: two-pass butterfly, weights-stationary pass 2, feature-major output.

Factor B = Bh @ Bl:
  Bl = stages 0..6  — block-diagonal over 8 contiguous 128-position blocks.
  Bh = stages 7..9  — mixes w = pos//128 across the 8 blocks, elementwise in
                      r = pos % 128.

Pass 1 (per 512-batch tile bt): T tiles in interleaved partition order.
  T[m][h] [128, 512]: partition p' = 32*wl + rl  <->  pos (32m + rl) + 128*(4h+wl)
  built by col-tiled quads (M=32, tile_position) with lhsT = Bl^T block slice,
  rhs = x block [128, 512]; one psum [128, 2, 512] per (bt, m), evicted to a
  resident T_big sbuf tile (bf16), alternating Scalar/Vector.

Pass 2 (per bt-pair p): out^T in feature-major, D stationary.
  psum[q, b] = sum_h D[m][h][:, qh-slice]^T @ T[m][h][bt]   (q = 32*wo' + rl)
  One DVE tensor_scalar_add per psum tile fuses the bias (per-partition
  column) and writes bf16 to the osb staging tile; stores ride the scalar
  engine's HWDGE ring (separate FIFO from the sync-engine loads).
  Host transposes the feature-major output back (free).

Extras: ~10 warm-up matmuls on a zeroed tile at t=0 keep the PE HAM
clock-gate open during the DMA lead-in; all loads are 8KB-per-partition
contiguous descriptors issued upfront on the sync ring.
"""

import os
import sys
import numpy as np

for _p in ("/opt/trn_rl_repo", os.path.expanduser("~/.axon_site/_ro/trn_rl_repo")):
    if os.path.isdir(_p) and _p not in sys.path:
        sys.path.insert(0, _p)

import concourse.bass as bass
import concourse.bacc as bacc
import concourse.mybir as mybir
from concourse import tile
from concourse.bass_utils import run_bass_kernel_spmd

import ml_dtypes

N_CORES = 8
BATCH = 32768
N = 1024
LOG_N = 10
BC = BATCH // N_CORES   # 4096 rows per core
BT = 512                # batch tile (pass 1)
NBT = BC // BT          # 8

_last_exec_time_ns = None
_nc_cache = None


def _apply_stages(m: np.ndarray, twiddle: np.ndarray, idxs) -> np.ndarray:
    """Apply butterfly stages `idxs` to the rows of m (batch of vectors)."""
    n = N
    for idx in idxs:
        s = 1 << idx
        g = n // (2 * s)
        t = twiddle[0, 0, idx].astype(np.float64).reshape(g, s, 2, 2)
        xr = m.reshape(-1, g, 2, s)
        m = np.einsum("grij,bgjr->bgir", t, xr).reshape(-1, n)
    return m


def _host_weights(twiddle: np.ndarray):
    eye = np.eye(N, dtype=np.float64)
    blt = _apply_stages(eye, twiddle, range(7))        # BlT[k, p] = Bl[p, k]
    bht = _apply_stages(eye, twiddle, range(7, 10))    # BhT[k, p] = Bh[p, k]

    # pass-1 lhsT: bl_pack[k, w, m, r32] = Bl[128w + 32m + r32, 128w + k]
    bl_pack = np.zeros((128, 8, 4, 32), dtype=np.float64)
    for w in range(8):
        blk = blt[128 * w:128 * (w + 1), 128 * w:128 * (w + 1)]  # [k, r]
        bl_pack[:, w] = blk.reshape(128, 4, 32)

    # pass-2 stationary operand: d_pack[p', m, h, q]
    #   p' = 32*wl + rl_in  -> pos_in  = 32m + rl_in + 128*(4h + wl)
    #   q  = 32*w_out + rl_out -> pos_out = 32m + rl_out + 128*w_out
    # value = BhT[pos_in, pos_out] = Bh[pos_out, pos_in]
    wl = np.arange(4)[:, None]          # [4, 1]
    rl = np.arange(32)[None, :]         # [1, 32]
    wo = np.arange(8)[:, None]
    d_pack = np.zeros((128, 4, 2, 256), dtype=np.float64)
    for m in range(4):
        for h in range(2):
            pos_in = (32 * m + rl + 128 * (4 * h + wl))        # [4, 32]
            pos_out = (32 * m + rl + 128 * wo)                 # [8, 32]
            # nonzero only when rl_in == rl_out
            sub = bht[np.ix_(pos_in.ravel(), pos_out.ravel())]  # [128, 256]
            mask = (rl.ravel()[None, :].repeat(4, 0).ravel()[:, None]
                    == rl.ravel()[None, :].repeat(8, 0).ravel()[None, :])
            d_pack[:, m, h, :] = np.where(mask, sub, 0.0)

    return bl_pack, d_pack


def _bias_cols(bias: np.ndarray) -> np.ndarray:
    # bias_col[p = 32*wo' + rl, g = 2m + qh] = bias[128*(4qh + wo') + 32m + rl]
    out = np.zeros((128, 8), dtype=np.float32)
    wo = np.arange(4)[:, None]
    rl = np.arange(32)[None, :]
    for m in range(4):
        for qh in range(2):
            pos = 128 * (4 * qh + wo) + 32 * m + rl   # [4, 32]
            out[:, 2 * m + qh] = bias[pos.ravel()].astype(np.float32)
    return np.ascontiguousarray(out)


def _build_nc():
    nc = bacc.Bacc("TRN2", target_bir_lowering=False)
    xtb = nc.dram_tensor("xtb", [NBT, 128, 8, BT], mybir.dt.bfloat16, kind="ExternalInput")
    bl = nc.dram_tensor("bl", [128, 8, 4, 32], mybir.dt.bfloat16, kind="ExternalInput")
    dd = nc.dram_tensor("dd", [128, 4, 2, 256], mybir.dt.bfloat16, kind="ExternalInput")
    bb = nc.dram_tensor("bb", [128, 8], mybir.dt.float32, kind="ExternalInput")
    out = nc.dram_tensor("out", [8, 128, 4, 2, BT], mybir.dt.bfloat16,
                         kind="ExternalOutput")

    with tile.TileContext(nc) as tc:
        with (
            tc.tile_pool(name="const", bufs=1) as cpool,
            # one shared psum pool: pass-1 units, pass-2 sweeps and warm-up
            # all use the same [128, 2, 512] f32 shape under one tag, so the
            # 4 bufs (8 banks) give 4-deep pipelining to whichever phase is
            # active instead of 2+2 split statically
            tc.tile_pool(name="psp", bufs=4, space="PSUM") as ps_pool,
        ):
            # warm-up source (zeros) — matmuls on it keep the PE busy so the
            # HAM clock-gate opens while the first x tiles stream in
            warm = cpool.tile([128, 512], mybir.dt.bfloat16)
            nc.gpsimd.memset(warm[:], 0)

            # x tiles alone on the sync HWDGE ring (nothing queues ahead of
            # them); consts ride the scalar ring in parallel
            xall = cpool.tile([128, NBT, 8, BT], mybir.dt.bfloat16)
            for g in range(NBT):
                nc.sync.dma_start(out=xall[:, g], in_=xtb[g])

            bls = cpool.tile([128, 8, 4, 32], mybir.dt.bfloat16)
            nc.scalar.dma_start(out=bls[:], in_=bl[:])
            dds = cpool.tile([128, 4, 2, 256], mybir.dt.bfloat16)
            nc.scalar.dma_start(out=dds[:], in_=dd[:])
            bbt = cpool.tile([128, 8], mybir.dt.float32)
            nc.scalar.dma_start(out=bbt[:], in_=bb[:])

            # warm-up matmuls (results discarded) bridge the PE from its
            # preamble (~7us) to the first x tile (~10us) so the HAM
            # clock-gate opens and stays open
            wps = ps_pool.tile([128, 2, 512], mybir.dt.float32, tag="ps")
            for _ in range(8):
                nc.tensor.matmul(wps[:, 0, :], warm[:, 0:128], warm[:],
                                 start=True, stop=True)

            # resident intermediate: T_big[p', m, h, bt, b]
            t_big = cpool.tile([128, 4, 2, NBT, BT], mybir.dt.bfloat16)
            # output staging, double-buffered over pair parity
            osb = cpool.tile([128, 8, 2, 2, BT], mybir.dt.bfloat16)

            # evictions are the co-bottleneck: 64 psum->sbuf units split
            # between ScalarE (~1.11us/unit) and VectorE (~1.25us/unit);
            # Bresenham split 34:30 balances their busy time
            evict_state = [0, 0]  # units seen, units given to ACT

            def use_act():
                evict_state[0] += 1
                target = (evict_state[0] * 34 + 32) // 64
                if evict_state[1] < target:
                    evict_state[1] += 1
                    return True
                return False

            def pass1_bt(bt):
                for m in range(4):
                    ps = ps_pool.tile([128, 2, 512], mybir.dt.float32, tag="ps")
                    for h in range(2):
                        for wl in range(4):
                            w = 4 * h + wl
                            nc.tensor.matmul(
                                ps[32 * wl:32 * (wl + 1), h, :],
                                bls[:, w, m, :],
                                xall[:, bt, w, :],
                                start=True,
                                stop=True,
                                tile_position=(0, 32 * wl),
                            )
                    if use_act():
                        nc.scalar.copy(out=t_big[:, m, :, bt, :], in_=ps[:])
                    else:
                        nc.vector.tensor_copy(out=t_big[:, m, :, bt, :], in_=ps[:])

            def pass2_phase(pi, bts):
                par = pi % 2
                nbc = len(bts)
                for m in range(4):
                    for qh in range(2):
                        g = 2 * m + qh
                        ps = ps_pool.tile([128, 2, 512], mybir.dt.float32, tag="ps")
                        for c, bt in enumerate(bts):
                            for h in range(2):
                                nc.tensor.matmul(
                                    ps[:, c, :],
                                    dds[:, m, h, 128 * qh:128 * (qh + 1)],
                                    t_big[:, m, h, bt, :],
                                    start=(h == 0),
                                    stop=(h == 1),
                                )
                        if use_act():
                            nc.scalar.activation(
                                osb[:, g, par, 0:nbc],
                                ps[:, 0:nbc, :],
                                mybir.ActivationFunctionType.Identity,
                                bias=bbt[:, g:g + 1],
                            )
                        else:
                            nc.vector.tensor_scalar_add(
                                osb[:, g, par, 0:nbc],
                                ps[:, 0:nbc, :],
                                bbt[:, g:g + 1],
                            )
                        # early stores ride the idle GPSIMD SWDGE queue; late
                        # stores use the sync ring (loads done by then) so the
                        # SWDGE drain doesn't sit on the kernel tail
                        eng = nc.gpsimd if pi < 2 else nc.sync
                        p_idx, c0 = bts[0] // 2, bts[0] % 2
                        eng.dma_start(
                            out=out[g][:, p_idx, c0:c0 + nbc],
                            in_=osb[:, g, par, 0:nbc],
                        )

            # interleave pass1 bt groups with pass2 sweeps so the in-order PE
            # program fills DMA-wait gaps with useful matmuls; the last two
            # phases are single-bt so the kernel tail is short
            for pi, bts in enumerate([(0, 1), (2, 3), (4, 5), (6,), (7,)]):
                for bt in bts:
                    pass1_bt(bt)
                pass2_phase(pi, bts)

    nc.compile()
    return nc


def kernel(x: np.ndarray, twiddle: np.ndarray, bias: np.ndarray) -> np.ndarray:
    global _last_exec_time_ns, _nc_cache

    bl_pack, d_pack = _host_weights(twiddle)
    bl_host = np.ascontiguousarray(bl_pack.astype(ml_dtypes.bfloat16))
    d_host = np.ascontiguousarray(d_pack.astype(ml_dtypes.bfloat16))
    bb_host = _bias_cols(np.asarray(bias))

    x = np.ascontiguousarray(x, dtype=np.float32)
    xb = x.astype(ml_dtypes.bfloat16)
    # [cores, NBT, 128 part, 8 w, BT] with tile g contiguous in HBM
    xtb_all = np.ascontiguousarray(
        xb.reshape(N_CORES, NBT, BT, 8, 128).transpose(0, 1, 4, 3, 2)
    )

    if _nc_cache is None:
        _nc_cache = _build_nc()
    nc = _nc_cache

    in_maps = [
        {"xtb": xtb_all[i], "bl": bl_host, "dd": d_host, "bb": bb_host}
        for i in range(N_CORES)
    ]

    trace = bool(int(os.environ.get("BUTTERFLY_TRACE", "0")))
    res = run_bass_kernel_spmd(
        nc,
        in_maps,
        core_ids=list(range(N_CORES)),
        trace=trace,
    )
    _last_exec_time_ns = res.exec_time_ns

    outs = []
    for i in range(N_CORES):
        o = np.asarray(res.results[i]["out"])  # [8 g, 128 q, 4096 b] bf16
        # g = 2m + qh, q = 32*wo' + rl; pos = 128*(4qh + wo') + 32m + rl
        o = o.astype(np.float32).reshape(4, 2, 4, 32, BC)
        o = o.transpose(4, 1, 2, 0, 3).reshape(BC, N)
        outs.append(o)
    return np.concatenate(outs, axis=0)


# revision 15
# speedup vs baseline: 1.4672x; 1.0173x over previous
"""v5: two-pass butterfly, weights-stationary pass 2, feature-major output.

Factor B = Bh @ Bl:
  Bl = stages 0..6  — block-diagonal over 8 contiguous 128-position blocks.
  Bh = stages 7..9  — mixes w = pos//128 across the 8 blocks, elementwise in
                      r = pos % 128.

Pass 1 (per 512-batch tile bt): T tiles in interleaved partition order.
  T[m][h] [128, 512]: partition p' = 32*wl + rl  <->  pos (32m + rl) + 128*(4h+wl)
  built by col-tiled quads (M=32, tile_position) with lhsT = Bl^T block slice,
  rhs = x block [128, 512]; one psum [128, 2, 512] per (bt, m), evicted to a
  resident T_big sbuf tile (bf16), alternating Scalar/Vector.

Pass 2 (per bt-pair p): out^T in feature-major, D stationary.
  psum[q, b] = sum_h D[m][h][:, qh-slice]^T @ T[m][h][bt]   (q = 32*wo' + rl)
  One DVE tensor_scalar_add per psum tile fuses the bias (per-partition
  column) and writes bf16 to the osb staging tile; stores ride the scalar
  engine's HWDGE ring (separate FIFO from the sync-engine loads).
  Host transposes the feature-major output back (free).

Extras: ~10 warm-up matmuls on a zeroed tile at t=0 keep the PE HAM
clock-gate open during the DMA lead-in; all loads are 8KB-per-partition
contiguous descriptors issued upfront on the sync ring.
"""

import os
import sys
import numpy as np

for _p in ("/opt/trn_rl_repo", os.path.expanduser("~/.axon_site/_ro/trn_rl_repo")):
    if os.path.isdir(_p) and _p not in sys.path:
        sys.path.insert(0, _p)

import concourse.bass as bass
import concourse.bacc as bacc
import concourse.mybir as mybir
from concourse import tile
from concourse.bass_utils import run_bass_kernel_spmd

import ml_dtypes

N_CORES = 8
BATCH = 32768
N = 1024
LOG_N = 10
BC = BATCH // N_CORES   # 4096 rows per core
BT = 512                # batch tile (pass 1)
NBT = BC // BT          # 8

_last_exec_time_ns = None
_nc_cache = None


def _apply_stages(m: np.ndarray, twiddle: np.ndarray, idxs) -> np.ndarray:
    """Apply butterfly stages `idxs` to the rows of m (batch of vectors)."""
    n = N
    for idx in idxs:
        s = 1 << idx
        g = n // (2 * s)
        t = twiddle[0, 0, idx].astype(np.float64).reshape(g, s, 2, 2)
        xr = m.reshape(-1, g, 2, s)
        m = np.einsum("grij,bgjr->bgir", t, xr).reshape(-1, n)
    return m


def _host_weights(twiddle: np.ndarray):
    eye = np.eye(N, dtype=np.float64)
    blt = _apply_stages(eye, twiddle, range(7))        # BlT[k, p] = Bl[p, k]
    bht = _apply_stages(eye, twiddle, range(7, 10))    # BhT[k, p] = Bh[p, k]

    # pass-1 lhsT: bl_pack[k, w, m, r32] = Bl[128w + 32m + r32, 128w + k]
    bl_pack = np.zeros((128, 8, 4, 32), dtype=np.float64)
    for w in range(8):
        blk = blt[128 * w:128 * (w + 1), 128 * w:128 * (w + 1)]  # [k, r]
        bl_pack[:, w] = blk.reshape(128, 4, 32)

    # pass-2 stationary operand: d_pack[p', m, h, q]
    #   p' = 32*wl + rl_in  -> pos_in  = 32m + rl_in + 128*(4h + wl)
    #   q  = 32*w_out + rl_out -> pos_out = 32m + rl_out + 128*w_out
    # value = BhT[pos_in, pos_out] = Bh[pos_out, pos_in]
    wl = np.arange(4)[:, None]          # [4, 1]
    rl = np.arange(32)[None, :]         # [1, 32]
    wo = np.arange(8)[:, None]
    d_pack = np.zeros((128, 4, 2, 256), dtype=np.float64)
    for m in range(4):
        for h in range(2):
            pos_in = (32 * m + rl + 128 * (4 * h + wl))        # [4, 32]
            pos_out = (32 * m + rl + 128 * wo)                 # [8, 32]
            # nonzero only when rl_in == rl_out
            sub = bht[np.ix_(pos_in.ravel(), pos_out.ravel())]  # [128, 256]
            mask = (rl.ravel()[None, :].repeat(4, 0).ravel()[:, None]
                    == rl.ravel()[None, :].repeat(8, 0).ravel()[None, :])
            d_pack[:, m, h, :] = np.where(mask, sub, 0.0)

    return bl_pack, d_pack


def _bias_cols(bias: np.ndarray) -> np.ndarray:
    # bias_col[p = 32*wo' + rl, g = 2m + qh] = bias[128*(4qh + wo') + 32m + rl]
    out = np.zeros((128, 8), dtype=np.float32)
    wo = np.arange(4)[:, None]
    rl = np.arange(32)[None, :]
    for m in range(4):
        for qh in range(2):
            pos = 128 * (4 * qh + wo) + 32 * m + rl   # [4, 32]
            out[:, 2 * m + qh] = bias[pos.ravel()].astype(np.float32)
    return np.ascontiguousarray(out)


def _build_nc():
    nc = bacc.Bacc("TRN2", target_bir_lowering=False)
    xtb = nc.dram_tensor("xtb", [NBT, 128, 8, BT], mybir.dt.bfloat16, kind="ExternalInput")
    bl = nc.dram_tensor("bl", [128, 8, 4, 32], mybir.dt.bfloat16, kind="ExternalInput")
    dd = nc.dram_tensor("dd", [128, 4, 2, 256], mybir.dt.bfloat16, kind="ExternalInput")
    bb = nc.dram_tensor("bb", [128, 8], mybir.dt.float32, kind="ExternalInput")
    out = nc.dram_tensor("out", [8, 128, 4, 2, BT], mybir.dt.bfloat16,
                         kind="ExternalOutput")

    with tile.TileContext(nc) as tc:
        with (
            tc.tile_pool(name="const", bufs=1) as cpool,
            # one shared psum pool: pass-1 units, pass-2 sweeps and warm-up
            # all use the same [128, 2, 512] f32 shape under one tag, so the
            # 4 bufs (8 banks) give 4-deep pipelining to whichever phase is
            # active instead of 2+2 split statically
            tc.tile_pool(name="psp", bufs=4, space="PSUM") as ps_pool,
        ):
            # warm-up source (zeros) — matmuls on it keep the PE busy so the
            # HAM clock-gate opens while the first x tiles stream in
            warm = cpool.tile([128, 512], mybir.dt.bfloat16)
            nc.gpsimd.memset(warm[:], 0)

            # sync ring: bls (gates the first quad) then the x tiles, x0 in
            # halves so the first quads start half a tile earlier; the other
            # consts ride the scalar ring in parallel
            bls = cpool.tile([128, 8, 4, 32], mybir.dt.bfloat16)
            nc.sync.dma_start(out=bls[:], in_=bl[:])
            xall = cpool.tile([128, NBT, 8, BT], mybir.dt.bfloat16)
            nc.sync.dma_start(out=xall[:, 0, 0:4], in_=xtb[0][:, 0:4])
            nc.sync.dma_start(out=xall[:, 0, 4:8], in_=xtb[0][:, 4:8])
            for g in range(1, NBT):
                nc.sync.dma_start(out=xall[:, g], in_=xtb[g])

            dds = cpool.tile([128, 4, 2, 256], mybir.dt.bfloat16)
            nc.scalar.dma_start(out=dds[:], in_=dd[:])
            bbt = cpool.tile([128, 8], mybir.dt.float32)
            nc.scalar.dma_start(out=bbt[:], in_=bb[:])

            # warm-up matmuls (results discarded) bridge the PE from its
            # preamble (~7us) to the first x tile (~11us) so the HAM
            # clock-gate opens and stays open
            wps = ps_pool.tile([128, 2, 512], mybir.dt.float32, tag="ps")
            for _ in range(11):
                nc.tensor.matmul(wps[:, 0, :], warm[:, 0:128], warm[:],
                                 start=True, stop=True)

            # resident intermediate: T_big[p', m, h, bt, b]
            t_big = cpool.tile([128, 4, 2, NBT, BT], mybir.dt.bfloat16)
            # output staging, double-buffered over pair parity
            osb = cpool.tile([128, 8, 2, 2, BT], mybir.dt.bfloat16)

            # evictions are the co-bottleneck: 64 psum->sbuf units split
            # between ScalarE (~1.11us/unit) and VectorE (~1.25us/unit);
            # Bresenham split 34:30 balances their busy time
            evict_state = [0, 0]  # units seen, units given to ACT

            def use_act():
                evict_state[0] += 1
                target = (evict_state[0] * 34 + 32) // 64
                if evict_state[1] < target:
                    evict_state[1] += 1
                    return True
                return False

            def pass1_bt(bt):
                for m in range(4):
                    ps = ps_pool.tile([128, 2, 512], mybir.dt.float32, tag="ps")
                    for h in range(2):
                        for wl in range(4):
                            w = 4 * h + wl
                            nc.tensor.matmul(
                                ps[32 * wl:32 * (wl + 1), h, :],
                                bls[:, w, m, :],
                                xall[:, bt, w, :],
                                start=True,
                                stop=True,
                                tile_position=(0, 32 * wl),
                            )
                    if use_act():
                        nc.scalar.copy(out=t_big[:, m, :, bt, :], in_=ps[:])
                    else:
                        nc.vector.tensor_copy(out=t_big[:, m, :, bt, :], in_=ps[:])

            def pass2_phase(pi, bts):
                par = pi % 2
                nbc = len(bts)
                for m in range(4):
                    for qh in range(2):
                        g = 2 * m + qh
                        ps = ps_pool.tile([128, 2, 512], mybir.dt.float32, tag="ps")
                        for c, bt in enumerate(bts):
                            for h in range(2):
                                nc.tensor.matmul(
                                    ps[:, c, :],
                                    dds[:, m, h, 128 * qh:128 * (qh + 1)],
                                    t_big[:, m, h, bt, :],
                                    start=(h == 0),
                                    stop=(h == 1),
                                )
                        if use_act():
                            nc.scalar.activation(
                                osb[:, g, par, 0:nbc],
                                ps[:, 0:nbc, :],
                                mybir.ActivationFunctionType.Identity,
                                bias=bbt[:, g:g + 1],
                            )
                        else:
                            nc.vector.tensor_scalar_add(
                                osb[:, g, par, 0:nbc],
                                ps[:, 0:nbc, :],
                                bbt[:, g:g + 1],
                            )
                        # early stores ride the idle GPSIMD SWDGE queue; late
                        # stores use the sync ring (loads done by then) so the
                        # SWDGE drain doesn't sit on the kernel tail
                        eng = nc.gpsimd if pi < 2 else nc.sync
                        p_idx, c0 = bts[0] // 2, bts[0] % 2
                        eng.dma_start(
                            out=out[g][:, p_idx, c0:c0 + nbc],
                            in_=osb[:, g, par, 0:nbc],
                        )

            # interleave pass1 bt groups with pass2 sweeps so the in-order PE
            # program fills DMA-wait gaps with useful matmuls; the last two
            # phases are single-bt so the kernel tail is short
            for pi, bts in enumerate([(0, 1), (2, 3), (4, 5), (6,), (7,)]):
                for bt in bts:
                    pass1_bt(bt)
                pass2_phase(pi, bts)

    nc.compile()
    return nc


def kernel(x: np.ndarray, twiddle: np.ndarray, bias: np.ndarray) -> np.ndarray:
    global _last_exec_time_ns, _nc_cache

    bl_pack, d_pack = _host_weights(twiddle)
    bl_host = np.ascontiguousarray(bl_pack.astype(ml_dtypes.bfloat16))
    d_host = np.ascontiguousarray(d_pack.astype(ml_dtypes.bfloat16))
    bb_host = _bias_cols(np.asarray(bias))

    x = np.ascontiguousarray(x, dtype=np.float32)
    xb = x.astype(ml_dtypes.bfloat16)
    # [cores, NBT, 128 part, 8 w, BT] with tile g contiguous in HBM
    xtb_all = np.ascontiguousarray(
        xb.reshape(N_CORES, NBT, BT, 8, 128).transpose(0, 1, 4, 3, 2)
    )

    if _nc_cache is None:
        _nc_cache = _build_nc()
    nc = _nc_cache

    in_maps = [
        {"xtb": xtb_all[i], "bl": bl_host, "dd": d_host, "bb": bb_host}
        for i in range(N_CORES)
    ]

    trace = bool(int(os.environ.get("BUTTERFLY_TRACE", "0")))
    res = run_bass_kernel_spmd(
        nc,
        in_maps,
        core_ids=list(range(N_CORES)),
        trace=trace,
    )
    _last_exec_time_ns = res.exec_time_ns

    outs = []
    for i in range(N_CORES):
        o = np.asarray(res.results[i]["out"])  # [8 g, 128 q, 4096 b] bf16
        # g = 2m + qh, q = 32*wo' + rl; pos = 128*(4qh + wo') + 32m + rl
        o = o.astype(np.float32).reshape(4, 2, 4, 32, BC)
        o = o.transpose(4, 1, 2, 0, 3).reshape(BC, N)
        outs.append(o)
    return np.concatenate(outs, axis=0)
